# revision 1
# baseline (speedup 1.0000x reference)
"""Trainium2 Bass kernel for nn_CrossModalFusionModel (sparse sliding-window
cross-attention, 2 modules: image<-text and text<-image).

Sharding: head-parallel tensor parallelism over 8 NeuronCores. Core h owns
attention head h (dh=128) of BOTH modules: it computes its head's Q/K/V with
host-folded projection weights (input-proj and attention-proj chains collapse
into one matmul), runs full-sequence banded attention for that head, and emits
a full-D o-projection partial plus its D-slice of the residual projection.
The host sums the 8 partials (the unshard step). No collectives.

Everything on-device lives in transposed [D, seq] layout so scores/AV/o-proj
chain through the PE without any on-chip transposes; the host pre-transposes
inputs and post-transposes outputs.

The reference's zero-pad attention slots (up to window tokens of kb/vb at the
sequence edges) collapse into ONE virtual pad column per query with
multiplicative weight n_pad(i), since all pad slots share the score q.kb.
"""

import math

import numpy as np
import ml_dtypes

N = 512          # tokens / patches
DM = 1024        # d_model
DH = 128         # head dim
NT = N // 128    # 4 j-tiles
C_IMG = 1024
C_TXT = 768
WINDOW = 64
NCORES = 8

# compute dtype knob: "f32r" | "f16" | "bf16" | "f32"
COMPUTE_DTYPE = "f32r"

_prog_cache = {}
LAST_RESULT = {}


def _mybir_cd(cd):
    from concourse import mybir
    return {
        "f32r": mybir.dt.float32r,
        "f16": mybir.dt.float16,
        "bf16": mybir.dt.bfloat16,
        "f32": mybir.dt.float32,
    }[cd]


def _np_cd(cd):
    return {
        "f32r": np.float32,
        "f16": np.float16,
        "bf16": ml_dtypes.bfloat16,
        "f32": np.float32,
    }[cd]


def _host_cd(x, cd):
    """Convert a float64 host array to the wire format of compute dtype cd."""
    x = np.ascontiguousarray(x)
    if cd == "f32r":
        x = x.astype(np.float32)
        hi = x.astype(ml_dtypes.bfloat16).astype(np.float32)
        lo = (x - hi).astype(ml_dtypes.bfloat16).astype(np.float32)
        return hi + lo
    return x.astype(_np_cd(cd))


def _build_program(cd):
    import concourse.bass as bass
    import concourse.tile as tile
    from concourse import bacc, mybir

    f32 = mybir.dt.float32
    CD = _mybir_cd(cd)
    Exp = mybir.ActivationFunctionType.Exp

    nc = bacc.Bacc("TRN2", target_bir_lowering=False, debug=False,
                   num_devices=NCORES)

    def din(name, shape, dt=CD):
        return nc.dram_tensor(name, shape, dt, kind="ExternalInput")

    def dout(name, shape, dt=f32):
        return nc.dram_tensor(name, shape, dt, kind="ExternalOutput")

    # Activations (transposed) and masks are identical on every core.
    xT_img = din("xT_img", [C_IMG, N])
    xT_txt = din("xT_txt", [C_TXT, N])
    maskM = din("maskM", [128, NT * N])
    maskP = din("maskP", [1, N])

    # Per-core (per-head) folded weights.
    mods = {}
    for m, cq, cc in (("ia", C_IMG, C_TXT), ("ta", C_TXT, C_IMG)):
        mods[m] = dict(
            wqT=din(f"wqT_{m}", [cq, DH]),
            wkT=din(f"wkT_{m}", [cc, DH]),
            wvT=din(f"wvT_{m}", [cc, DH]),
            woT=din(f"woT_{m}", [DH, DM]),
            kbcol=din(f"kbcol_{m}", [DH, 1]),
            vbpad=din(f"vbpad_{m}", [1, DH]),
            bvrow=din(f"bvrow_{m}", [1, DH]),
            bq=din(f"bq_{m}", [DH, 1], f32),
            bk=din(f"bk_{m}", [DH, 1], f32),
            po=dout(f"po_{m}", [DM, N]),
            xr=dout(f"xr_{m}", [DH, N]),
        )
    rwT_img = din("rwT_img", [C_IMG, DH])   # ip_w D-slice (residual)
    rwT_txt = din("rwT_txt", [C_TXT, DH])   # tp_w D-slice
    brx = din("brx", [DH, 1], f32)          # ip_b slice
    brt = din("brt", [DH, 1], f32)          # tp_b slice
    ones_c = din("ones_c", [128, 1])
    ones_r = din("ones_r", [1, 128])

    with tile.TileContext(nc) as tc:
        with tc.tile_pool(name="consts", bufs=1) as consts, \
             tc.tile_pool(name="work", bufs=3) as work, \
             tc.tile_pool(name="epool", bufs=3) as epool, \
             tc.tile_pool(name="small", bufs=2) as small, \
             tc.tile_pool(name="ps_st", bufs=2, space="PSUM") as ps_st, \
             tc.tile_pool(name="ps_small", bufs=1, space="PSUM") as ps_small, \
             tc.tile_pool(name="ps_acc", bufs=4, space="PSUM") as ps_acc:

            def load3(name, dram, c, n):
                t = consts.tile([128, c // 128, n], CD, tag=name)
                nc.sync.dma_start(
                    t[:], dram.ap().rearrange("(c p) n -> p c n", p=128))
                return t

            xi = load3("xi", xT_img, C_IMG, N)
            xt = load3("xt", xT_txt, C_TXT, N)
            rwi = load3("rwi", rwT_img, C_IMG, DH)
            rwt = load3("rwt", rwT_txt, C_TXT, DH)

            mM = consts.tile([128, NT * N], CD, tag="mM")
            nc.sync.dma_start(mM[:], maskM[:])
            mP = consts.tile([1, N], CD, tag="mP")
            nc.sync.dma_start(mP[:], maskP[:])

            ones_col = consts.tile([128, 1], CD, tag="ones_col")
            nc.sync.dma_start(ones_col[:], ones_c[:])
            ones_row = consts.tile([1, 128], CD, tag="ones_row")
            nc.sync.dma_start(ones_row[:], ones_r[:])

            sb = {}
            for m, cq, cc in (("ia", C_IMG, C_TXT), ("ta", C_TXT, C_IMG)):
                d = mods[m]
                sb[m] = dict(
                    wq=load3(f"wq_{m}", d["wqT"], cq, DH),
                    wk=load3(f"wk_{m}", d["wkT"], cc, DH),
                    wv=load3(f"wv_{m}", d["wvT"], cc, DH),
                )
                wo = consts.tile([DH, DM], CD, tag=f"wo_{m}")
                nc.sync.dma_start(wo[:], d["woT"][:])
                kbc = consts.tile([DH, 1], CD, tag=f"kbc_{m}")
                nc.sync.dma_start(kbc[:], d["kbcol"][:])
                vbp = consts.tile([1, DH], CD, tag=f"vbp_{m}")
                nc.sync.dma_start(vbp[:], d["vbpad"][:])
                bvr = consts.tile([1, DH], CD, tag=f"bvr_{m}")
                nc.sync.dma_start(bvr[:], d["bvrow"][:])
                bq = consts.tile([DH, 1], f32, tag=f"bq_{m}")
                nc.sync.dma_start(bq[:], d["bq"][:])
                bk = consts.tile([DH, 1], f32, tag=f"bk_{m}")
                nc.sync.dma_start(bk[:], d["bk"][:])
                sb[m].update(wo=wo, kbc=kbc, vbp=vbp, bvr=bvr, bq=bq, bk=bk)
            bxi = consts.tile([DH, 1], f32, tag="bxi")
            nc.sync.dma_start(bxi[:], brx[:])
            bxt = consts.tile([DH, 1], f32, tag="bxt")
            nc.sync.dma_start(bxt[:], brt[:])

            def projT(w3, x3, nct, bias_col, tag, out_dt=CD):
                """out^T [128, N] = (x @ W^T)^T + bias, via contraction tiles."""
                ps = ps_acc.tile([128, N], f32, tag="acc")
                for ct in range(nct):
                    nc.tensor.matmul(ps[:], w3[:, ct, :], x3[:, ct, :],
                                     start=(ct == 0), stop=(ct == nct - 1))
                out = work.tile([128, N], out_dt, tag="sb_" + tag)
                nc.vector.tensor_scalar_add(out[:], ps[:], bias_col[:])
                return out

            def vproj_nat(x3, w3, nct, bvr, tag):
                """V natural [j, d] in one [128, NT*128] tile (jt at free jt*128)."""
                ps = ps_acc.tile([128, NT * DH], f32, tag="acc")
                for jt in range(NT):
                    blk = ps[:, jt * DH:(jt + 1) * DH]
                    for ct in range(nct):
                        nc.tensor.matmul(
                            blk, x3[:, ct, jt * 128:(jt + 1) * 128],
                            w3[:, ct, :], start=(ct == 0), stop=False)
                    nc.tensor.matmul(blk, ones_row[:, :], bvr[:],
                                     start=False, stop=True)
                out = work.tile([128, NT * DH], CD, tag="sb_" + tag)
                nc.vector.tensor_copy(out[:], ps[:])
                return out

            def residT(w3, x3, nct, bias_col, dram, tag):
                ps = ps_acc.tile([128, N], f32, tag="acc")
                for ct in range(nct):
                    nc.tensor.matmul(ps[:], w3[:, ct, :], x3[:, ct, :],
                                     start=(ct == 0), stop=(ct == nct - 1))
                out = work.tile([128, N], f32, tag="sb_" + tag)
                nc.vector.tensor_scalar_add(out[:], ps[:], bias_col[:])
                nc.sync.dma_start(dram[:], out[:])

            for m, xq3, nq, xc3, ncc in (("ia", xi, 8, xt, 6),
                                         ("ta", xt, 6, xi, 8)):
                s = sb[m]
                d = mods[m]
                qT = projT(s["wq"], xq3, nq, s["bq"], "q")
                kT = projT(s["wk"], xc3, ncc, s["bk"], "k")
                vN = vproj_nat(xc3, s["wv"], ncc, s["bvr"], "v")

                # scores S^T per j-tile, exp, band-mask
                eTm = epool.tile([128, NT * N], CD, tag="eTm")
                for jt in range(NT):
                    st = ps_st.tile([128, N], f32, tag="st")
                    nc.tensor.matmul(st[:], kT[:, jt * 128:(jt + 1) * 128],
                                     qT[:], start=True, stop=True)
                    eT = epool.tile([128, N], CD, tag="eT")
                    nc.scalar.activation(eT[:], st[:], Exp)
                    nc.vector.tensor_mul(eTm[:, jt * N:(jt + 1) * N], eT[:],
                                         mM[:, jt * N:(jt + 1) * N])
                # virtual pad column (score q.kb, weight n_pad)
                sp = ps_small.tile([1, N], f32, tag="smallp")
                nc.tensor.matmul(sp[:], s["kbc"][:], qT[:], start=True,
                                 stop=True)
                eP = small.tile([1, N], CD, tag="eP")
                nc.scalar.activation(eP[:], sp[:], Exp)
                ePm = small.tile([1, N], CD, tag="ePm")
                nc.vector.tensor_mul(ePm[:], eP[:], mP[:])

                # softmax denominators
                ssum = ps_small.tile([1, N], f32, tag="smallp2")
                for jt in range(NT):
                    nc.tensor.matmul(ssum[:], ones_col[:],
                                     eTm[:, jt * N:(jt + 1) * N],
                                     start=(jt == 0), stop=False)
                nc.tensor.matmul(ssum[:], ones_col[0:1, :], ePm[:],
                                 start=False, stop=True)
                rinv = small.tile([1, N], CD, tag="rinv")
                with nc.allow_low_precision(
                        reason="softmax 1/denom feeds a CD-dtype matmul; "
                               "CD is >= fp16 and denom is O(1-100)"):
                    nc.vector.reciprocal(rinv[:], ssum[:])

                # O^T = V^T E^T (+ pad)
                oT = ps_acc.tile([128, N], f32, tag="acc")
                for jt in range(NT):
                    nc.tensor.matmul(oT[:], vN[:, jt * DH:(jt + 1) * DH],
                                     eTm[:, jt * N:(jt + 1) * N],
                                     start=(jt == 0), stop=False)
                nc.tensor.matmul(oT[:], s["vbp"][:], ePm[:], start=False,
                                 stop=True)

                # normalize: broadcast rinv to 128 partitions via PE
                rbc = ps_acc.tile([128, N], f32, tag="acc")
                nc.tensor.matmul(rbc[:], ones_row[:], rinv[:], start=True,
                                 stop=True)
                rbc_sb = work.tile([128, N], f32, tag="rbc_sb")
                nc.vector.tensor_copy(rbc_sb[:], rbc[:])
                onorm = work.tile([128, N], CD, tag="onorm")
                nc.vector.tensor_mul(onorm[:], oT[:], rbc_sb[:])

                # o-projection partial: po[dt*128:, :] = wo_h[:, dt].T @ onorm
                for dt_i in range(DM // 128):
                    po = ps_acc.tile([128, N], f32, tag="acc")
                    nc.tensor.matmul(po[:],
                                     s["wo"][:, dt_i * 128:(dt_i + 1) * 128],
                                     onorm[:], start=True, stop=True)
                    po_sb = work.tile([128, N], f32, tag="po_sb")
                    nc.vector.tensor_copy(po_sb[:], po[:])
                    nc.sync.dma_start(d["po"][dt_i * 128:(dt_i + 1) * 128, :],
                                      po_sb[:])

            # residual D-slices (fp32 out)
            residT(rwi, xi, 8, bxi, mods["ia"]["xr"], "xri")
            residT(rwt, xt, 6, bxt, mods["ta"]["xr"], "xrt")

    nc.compile()
    return nc


def _masks(cd):
    i = np.arange(N)
    j = np.arange(N)
    band = (j[:, None] >= i[None, :] - WINDOW // 2) & \
           (j[:, None] <= i[None, :] + WINDOW // 2 + 1)   # [j, i]
    length = band.sum(axis=0)
    npad = np.maximum(0, WINDOW - length)
    mM = band.astype(np.float64).reshape(NT, 128, N).transpose(1, 0, 2) \
             .reshape(128, NT * N)
    mP = npad.astype(np.float64)[None, :]
    return _host_cd(mM, cd), _host_cd(mP, cd)


def kernel(**inputs):
    from concourse.bass_utils import run_bass_kernel_spmd

    cd = COMPUTE_DTYPE
    if cd not in _prog_cache:
        _prog_cache[cd] = _build_program(cd)
    nc = _prog_cache[cd]

    f8 = lambda x: np.asarray(x, dtype=np.float64)
    images = f8(inputs["images"])[0]        # [N, 1024]
    caps = f8(inputs["capitions"])[0]       # [N, 768]
    ip_w, ip_b = f8(inputs["ip_w"]), f8(inputs["ip_b"])
    tp_w, tp_b = f8(inputs["tp_w"]), f8(inputs["tp_b"])

    sc = 1.0 / math.sqrt(DH)
    mM, mP = _masks(cd)
    xTi = _host_cd(images.T, cd)
    xTt = _host_cd(caps.T, cd)

    in_maps = []
    for h in range(NCORES):
        sl = slice(h * DH, (h + 1) * DH)
        im = {
            "xT_img": xTi, "xT_txt": xTt, "maskM": mM, "maskP": mP,
            "rwT_img": _host_cd(ip_w[sl].T, cd),
            "rwT_txt": _host_cd(tp_w[sl].T, cd),
            "ones_c": _host_cd(np.ones((128, 1)), cd),
            "ones_r": _host_cd(np.ones((1, 128)), cd),
            "brx": np.ascontiguousarray(ip_b[sl, None], dtype=np.float32),
            "brt": np.ascontiguousarray(tp_b[sl, None], dtype=np.float32),
        }
        for m, pw, pb, cw, cb in (("ia", ip_w, ip_b, tp_w, tp_b),
                                  ("ta", tp_w, tp_b, ip_w, ip_b)):
            qw, qb = f8(inputs[f"{m}_qw"]), f8(inputs[f"{m}_qb"])
            kw, kb = f8(inputs[f"{m}_kw"]), f8(inputs[f"{m}_kb"])
            vw, vb = f8(inputs[f"{m}_vw"]), f8(inputs[f"{m}_vb"])
            ow = f8(inputs[f"{m}_ow"])
            im[f"wqT_{m}"] = _host_cd(((qw[sl] @ pw) * sc).T, cd)
            im[f"bq_{m}"] = ((qw[sl] @ pb + qb[sl]) * sc)[:, None] \
                .astype(np.float32)
            im[f"wkT_{m}"] = _host_cd((kw[sl] @ cw).T, cd)
            im[f"bk_{m}"] = (kw[sl] @ cb + kb[sl])[:, None].astype(np.float32)
            im[f"wvT_{m}"] = _host_cd((vw[sl] @ cw).T, cd)
            im[f"bvrow_{m}"] = _host_cd((vw[sl] @ cb + vb[sl])[None, :], cd)
            im[f"woT_{m}"] = _host_cd(ow[:, sl].T, cd)
            im[f"kbcol_{m}"] = _host_cd(kb[sl, None], cd)
            im[f"vbpad_{m}"] = _host_cd(vb[sl][None, :], cd)
        in_maps.append(im)

    res = run_bass_kernel_spmd(nc, in_maps, list(range(NCORES)))
    LAST_RESULT["res"] = res

    outs = []
    for m in ("ia", "ta"):
        acc = np.zeros((DM, N), dtype=np.float64)
        for h in range(NCORES):
            r = res.results[h]
            acc += r[f"po_{m}"].astype(np.float64)
            acc[h * DH:(h + 1) * DH] += r[f"xr_{m}"].astype(np.float64)
        acc += f8(inputs["ia_ob" if m == "ia" else "ta_ob"])[:, None]
        outs.append(np.ascontiguousarray(acc.T[None]).astype(np.float32))
    return outs[0], outs[1]



# revision 27
# speedup vs baseline: 1.7456x; 1.7456x over previous
"""Trainium2 Bass kernel for nn_CrossModalFusionModel (sparse sliding-window
cross-attention, 2 modules: image<-text and text<-image).

Sharding: head-parallel tensor parallelism over 8 NeuronCores. Core h owns
attention head h (dh=128) of BOTH modules: it computes its head's Q/K/V with
host-folded projection weights (input-proj and attention-proj chains collapse
into one matmul), runs banded sliding-window attention for that head, and
emits a full-D o-projection partial (f16). The host sums the 8 partials.
No collectives.

v2 structure (per module):
 - Q^T,K^T [dh, N] and V natural [j, dh] via contraction-tiled matmuls.
 - Scores computed NATURALLY per 128-query i-tile against a 193-wide context
   window (the 66-token band + tile skew), additive -30k mask, then one ACT
   exp with accum_out -> masked exp AND softmax denominator in one pass.
 - Normalization is a per-partition tensor_scalar multiply (1/denom column),
   BEFORE the PE transpose to [j, i] layout, so AV directly yields the
   normalized context sum.
 - The reference's zero-pad slots collapse into one virtual column: weight
   npad(i) folds in as an ln(npad) activation bias; pad exists only for edge
   i-tiles 0 and 3.
 - V carries NO bias: since softmax weights sum to 1, the real-slot v bias
   (vw@cb+vb) is added once to the attention output column; the pad slots'
   missing vw@cb is restored by a rank-1 correction -(vw@cb) x A_pad on edge
   tiles.
 - o-projection partial po[DM, N] in f16; wo columns are rotated per-core so
   tile 0 is the core's own D-slice, letting the device fold the residual
   (input-projection slice) into po tile 0. Host un-rotates and sums.
"""

import math

import numpy as np
import ml_dtypes

N = 512          # tokens / patches
DM = 1024        # d_model
DH = 128         # head dim
NT = N // 128    # 4 j-tiles
C_IMG = 1024
C_TXT = 768
WINDOW = 64
NCORES = 8
W_IT = (256, 384, 384, 256)  # banded window width per 128-query i-tile
JT0 = (0, 0, 1, 2)           # first context j-tile of each window
MOFF = (0, 256, 640, 1024)   # mask column offset per i-tile
MW = 1280                    # total mask columns
EDGE = {0: 0, 3: 1}          # i-tile -> lnpad column (pad slots exist here)

# compute dtype knob: "f16" | "bf16" | "f32"
COMPUTE_DTYPE = "f16"

_prog_cache = {}
LAST_RESULT = {}

def _mybir_cd(cd):
    from concourse import mybir
    return {
        "f16": mybir.dt.float16,
        "bf16": mybir.dt.bfloat16,
        "f32": mybir.dt.float32,
    }[cd]


def _np_cd(cd):
    return {
        "f16": np.float16,
        "bf16": ml_dtypes.bfloat16,
        "f32": np.float32,
    }[cd]


def _host_cd(x, cd):
    """Convert a float64 host array to the wire format of compute dtype cd."""
    x = np.ascontiguousarray(x)
    return x.astype(_np_cd(cd))


def _build_program(cd):
    import concourse.bass as bass
    import concourse.tile as tile
    from concourse import bacc, mybir

    f32 = mybir.dt.float32
    CD = _mybir_cd(cd)
    Exp = mybir.ActivationFunctionType.Exp

    nc = bacc.Bacc("TRN2", target_bir_lowering=False, debug=False,
                   num_devices=NCORES)

    def din(name, shape, dt=CD):
        return nc.dram_tensor(name, shape, dt, kind="ExternalInput")

    def dout(name, shape, dt=CD):
        return nc.dram_tensor(name, shape, dt, kind="ExternalOutput")

    # Inputs are coalesced into partition-major packs: every DMA instruction
    # pays a fixed HWDGE descriptor-generation cost, so few big linear
    # transfers beat many small ones. Layouts are prepared host-side.
    g_wq_ia = din("g_wq_ia", [128, 8 * DH])       # folded q weights, mod ia
    g_xi = din("g_xi", [128, 8 * N])              # images^T
    g_kv_ta = din("g_kv_ta", [128, 16 * DH])      # wk_ta | wv_ta
    g_wk_ia = din("g_wk_ia", [128, 6 * DH])
    g_xt = din("g_xt", [128, 6 * N])              # captions^T
    # f16 pack: maskB | identity | kbcol_ia | kbcol_ta | negc_ia | negc_ta
    g_pk16 = din("g_pk16", [128, MW + 128 + 2 + 256])
    g_qv_ia = din("g_qv_ia", [128, 12 * DH])      # wq_ta | wv_ia
    g_rw_ia = din("g_rw_ia", [128, 8 * DH + DM])  # rw_img | wo_ia (rotated)
    g_rw_ta = din("g_rw_ta", [128, 6 * DH + DM])  # rw_txt | wo_ta (rotated)
    # f32 pack: bq_ia bk_ia bv_ia bq_ta bk_ta bv_ta brx brt lnpad(2)
    g_pf32 = din("g_pf32", [128, 10], f32)

    mods = {}
    for m in ("ia", "ta"):
        mods[m] = dict(po=dout(f"po_{m}", [128, 8 * N]))

    with tile.TileContext(nc) as tc:
        with tc.tile_pool(name="consts", bufs=1) as consts, \
             tc.tile_pool(name="work", bufs=1) as work, \
             tc.tile_pool(name="attn", bufs=2) as attn, \
             tc.tile_pool(name="small", bufs=2) as small, \
             tc.tile_pool(name="ps_acc", bufs=3, space="PSUM") as ps_acc, \
             tc.tile_pool(name="ps_sc", bufs=2, space="PSUM") as ps_sc, \
             tc.tile_pool(name="ps_eT", bufs=2, space="PSUM") as ps_eT, \
             tc.tile_pool(name="ps_oT", bufs=1, space="PSUM") as ps_oT:

            def gload(name, dram, cols, dt=CD, split=1):
                t = consts.tile([128, cols], dt, tag=name, name=name)
                step = cols // split
                for i in range(split):
                    nc.sync.dma_start(t[:, i * step:(i + 1) * step],
                                      dram[:, i * step:(i + 1) * step])
                return t

            def as3(t, lo, c, n):
                return t[:, lo:lo + c * n].rearrange("p (c n) -> p c n", c=c)

            # --- DMA loads in need-order (queue order ~ emission order) ---
            sb = {m: {} for m in ("ia", "ta")}
            t_wq_ia = gload("wq_ia", g_wq_ia, 8 * DH)
            sb["ia"]["wq"] = as3(t_wq_ia, 0, 8, DH)
            t_xi = gload("xi", g_xi, 8 * N, split=4)
            xi = as3(t_xi, 0, 8, N)
            pf32 = gload("pf32", g_pf32, 10, f32)
            sb["ia"]["bq"] = pf32[:, 0:1]
            sb["ia"]["bk"] = pf32[:, 1:2]
            sb["ia"]["bv"] = pf32[:, 2:3]
            sb["ta"]["bq"] = pf32[:, 3:4]
            sb["ta"]["bk"] = pf32[:, 4:5]
            sb["ta"]["bv"] = pf32[:, 5:6]
            bxi = pf32[:, 6:7]
            bxt = pf32[:, 7:8]
            lnp = pf32[:, 8:10]
            t_kv_ta = gload("kv_ta", g_kv_ta, 16 * DH)
            sb["ta"]["wk"] = as3(t_kv_ta, 0, 8, DH)
            sb["ta"]["wv"] = as3(t_kv_ta, 8 * DH, 8, DH)
            t_wk_ia = gload("wk_ia", g_wk_ia, 6 * DH)
            sb["ia"]["wk"] = as3(t_wk_ia, 0, 6, DH)
            t_xt = gload("xt", g_xt, 6 * N, split=2)
            xt = as3(t_xt, 0, 6, N)
            pk16 = gload("pk16", g_pk16, MW + 128 + 2 + 256)
            mB = pk16[:, 0:MW]
            idn = pk16[:, MW:MW + 128]
            sb["ia"]["kbc"] = pk16[:, MW + 128:MW + 129]
            sb["ta"]["kbc"] = pk16[:, MW + 129:MW + 130]
            sb["ia"]["ngc"] = pk16[0:1, MW + 130:MW + 258]
            sb["ta"]["ngc"] = pk16[0:1, MW + 258:MW + 386]
            t_qv_ia = gload("qv_ia", g_qv_ia, 12 * DH)
            sb["ta"]["wq"] = as3(t_qv_ia, 0, 6, DH)
            sb["ia"]["wv"] = as3(t_qv_ia, 6 * DH, 6, DH)
            t_rw_ia = gload("rw_ia", g_rw_ia, 8 * DH + DM)
            rwi = as3(t_rw_ia, 0, 8, DH)
            sb["ia"]["wo"] = t_rw_ia[:, 8 * DH:8 * DH + DM]
            t_rw_ta = gload("rw_ta", g_rw_ta, 6 * DH + DM)
            rwt = as3(t_rw_ta, 0, 6, DH)
            sb["ta"]["wo"] = t_rw_ta[:, 6 * DH:6 * DH + DM]

            # --- compute helpers ---
            def projT(w3, x3, nct, bias_col, tag, eng="act"):
                """out^T [128, N] = (x @ W^T)^T + bias."""
                ps = ps_acc.tile([128, N], f32, tag="acc", name="ps_" + tag)
                for ct in range(nct):
                    nc.tensor.matmul(ps[:], w3[:, ct, :], x3[:, ct, :],
                                     start=(ct == 0), stop=(ct == nct - 1))
                out = work.tile([128, N], CD, tag="sb_" + tag,
                                name="sb_" + tag)
                if eng == "act":
                    nc.scalar.add(out[:], ps[:], bias_col)
                else:
                    nc.vector.tensor_scalar_add(out[:], ps[:], bias_col[:])
                return out

            def vproj(x3, w3, nct, tag):
                """V natural [j, d], one [128, NT*DH] tile, NO bias."""
                ps = ps_acc.tile([128, NT * DH], f32, tag="acc",
                                 name="ps_" + tag)
                for jt in range(NT):
                    blk = ps[:, jt * DH:(jt + 1) * DH]
                    for ct in range(nct):
                        nc.tensor.matmul(
                            blk, x3[:, ct, jt * 128:(jt + 1) * 128],
                            w3[:, ct, :], start=(ct == 0),
                            stop=(ct == nct - 1))
                out = work.tile([128, NT * DH], CD, tag="sb_" + tag,
                                name="sb_" + tag)
                nc.vector.tensor_copy(out[:], ps[:])
                return out

            def attn_scores(m, it, qT, kT):
                """Banded scores -> masked exp -> normalized e (natural)."""
                s = sb[m]
                w = W_IT[it]
                j0 = JT0[it] * 128
                qsl = qT[:, it * 128:(it + 1) * 128]
                edge = it in EDGE
                st = ps_sc.tile([128, 385], f32, tag="st", name="st")
                nc.tensor.matmul(st[:, 0:w], qsl, kT[:, j0:j0 + w],
                                 start=True, stop=True)
                if edge:
                    nc.tensor.matmul(st[:, 384:385], qsl, s["kbc"][:],
                                     start=True, stop=True)
                sm = attn.tile([128, 384], CD, tag="sm", name="sm")
                nc.vector.tensor_add(sm[:, 0:w], st[:, 0:w],
                                     mB[:, MOFF[it]:MOFF[it] + w])
                e = attn.tile([128, 384], CD, tag="e", name="e")
                den = small.tile([128, 1], f32, tag="den", name="den")
                nc.scalar.activation(e[:, 0:w], sm[:, 0:w], Exp,
                                     accum_out=den[:])
                apad = None
                if edge:
                    epd = small.tile([128, 1], f32, tag="epd", name="epd")
                    nc.scalar.activation(
                        epd[:], st[:, 384:385], Exp,
                        bias=lnp[:, EDGE[it]:EDGE[it] + 1])
                    den2 = small.tile([128, 1], f32, tag="den2", name="den2")
                    nc.vector.tensor_add(den2[:], den[:], epd[:])
                    den = den2
                rinv = small.tile([128, 1], f32, tag="rinv", name="rinv")
                nc.vector.reciprocal(rinv[:], den[:])
                en = attn.tile([128, 384], CD, tag="en", bufs=4, name="en")
                nc.gpsimd.tensor_scalar_mul(en[:, 0:w], e[:, 0:w], rinv[:])
                if edge:
                    apad = small.tile([128, 1], CD, tag="apad", name="apad")
                    nc.gpsimd.tensor_mul(apad[:], epd[:], rinv[:])
                return en, apad

            def attn_transp(m, it, en, apad):
                """PE-transpose normalized e (and pad row) for this i-tile."""
                w = W_IT[it]
                eTp = ps_eT.tile([128, 512], CD, tag="eT", name="eTp")
                for t in range(w // 128):
                    nc.tensor.transpose(eTp[:, t * 128:(t + 1) * 128],
                                        en[:, t * 128:(t + 1) * 128], idn[:])
                if apad is not None:
                    nc.tensor.transpose(eTp[0:1, 384:512], apad[:], idn[:])
                return eTp

            def attn_ecopy(m, it, eTp, edge):
                w = W_IT[it]
                eT = attn.tile([128, 384], CD, tag="eTsb", name="eT")
                nc.vector.tensor_copy(eT[:, 0:w], eTp[:, 0:w])
                apr = None
                if edge:
                    apr = small.tile([1, 128], CD, tag="apr", name="apr")
                    nc.vector.tensor_copy(apr[:], eTp[0:1, 384:512])
                return eT, apr

            def attn_av(m, it, eT, apr, vN, oT):
                """Banded AV into the oT region for this i-tile."""
                s = sb[m]
                njt = W_IT[it] // 128
                region = oT[:, it * 128:(it + 1) * 128]
                nmm = njt + (1 if apr is not None else 0)
                for t in range(njt):
                    jt = JT0[it] + t
                    nc.tensor.matmul(
                        region, vN[:, jt * DH:(jt + 1) * DH],
                        eT[:, t * 128:(t + 1) * 128],
                        start=(t == 0), stop=(t == nmm - 1))
                if apr is not None:
                    nc.tensor.matmul(region, s["ngc"][:], apr[:],
                                     start=False, stop=True)

            def avstage(m, s1, vN, oT):
                """Interleave transposes, DVE copies and AV to keep PE dense."""
                pend = []
                for it in range(NT):
                    en, apad = s1[it]
                    eTp = attn_transp(m, it, en, apad)
                    eT, apr = attn_ecopy(m, it, eTp, apad is not None)
                    pend.append((it, eT, apr))
                    if len(pend) >= 2:
                        pit, peT, papr = pend.pop(0)
                        attn_av(m, pit, peT, papr, vN, oT)
                for pit, peT, papr in pend:
                    attn_av(m, pit, peT, papr, vN, oT)

            def po_step(m, po_all, onorm, xr, dt):
                """One o-projection D-tile: matmul, copy out, staged DMA."""
                s = sb[m]
                pp = ps_acc.tile([128, N], f32, tag="acc", name="pp")
                nc.tensor.matmul(pp[:], s["wo"][:, dt * 128:(dt + 1) * 128],
                                 onorm[:], start=True, stop=True)
                dst = po_all[:, dt * N:(dt + 1) * N]
                if dt == 0:
                    nc.vector.tensor_add(dst, pp[:], xr[:])
                elif dt in (2, 6):
                    nc.vector.tensor_copy(dst, pp[:])
                else:
                    nc.scalar.copy(dst, pp[:])
                if dt == 3:
                    nc.sync.dma_start(mods[m]["po"][:, 0:4 * N],
                                      po_all[:, 0:4 * N])
                elif dt == 5:
                    nc.sync.dma_start(mods[m]["po"][:, 4 * N:6 * N],
                                      po_all[:, 4 * N:6 * N])
                elif dt == 7:
                    nc.sync.dma_start(mods[m]["po"][:, 6 * N:8 * N],
                                      po_all[:, 6 * N:8 * N])

            def oproj(m, onorm, xr):
                po_all = work.tile([128, 8 * N], CD, tag="po_all", bufs=2,
                                   name="po_all")
                for dt in range(DM // 128):
                    po_step(m, po_all, onorm, xr, dt)

            # --- emission schedule (PE kept dense; xi-only work first so
            # scores wait neither on xt DMA nor leave PE idle) ---
            q_i = projT(sb["ia"]["wq"], xi, 8, sb["ia"]["bq"], "q_ia")
            k_t = projT(sb["ta"]["wk"], xi, 8, sb["ta"]["bk"], "k_ta")
            v_t = vproj(xi, sb["ta"]["wv"], 8, "v_ta")
            k_i = projT(sb["ia"]["wk"], xt, 6, sb["ia"]["bk"], "k_ia")
            ia_s1 = [attn_scores("ia", it, q_i, k_i) for it in range(NT)]
            q_t = projT(sb["ta"]["wq"], xt, 6, sb["ta"]["bq"], "q_ta")
            v_i = vproj(xt, sb["ia"]["wv"], 6, "v_ia")

            # ia AV stage interleaved with ta scores: ta's DVE/ACT chains
            # overlap ia's o-projection, and PE never waits on eT copies.
            oT_i = ps_oT.tile([128, N], f32, tag="oT", name="oT_i")
            ta_s1 = []
            pend = []
            for it in range(NT):
                en, apad = ia_s1[it]
                eTp = attn_transp("ia", it, en, apad)
                pend.append((it,) + attn_ecopy("ia", it, eTp, apad is not None))
                ta_s1.append(attn_scores("ta", it, q_t, k_t))
                if len(pend) >= 2:
                    pit, peT, papr = pend.pop(0)
                    attn_av("ia", pit, peT, papr, v_i, oT_i)
            for pit, peT, papr in pend:
                attn_av("ia", pit, peT, papr, v_i, oT_i)
            onorm_i = work.tile([128, N], CD, tag="onorm", bufs=2,
                                name="onorm_i")
            nc.vector.tensor_scalar_add(onorm_i[:], oT_i[:], sb["ia"]["bv"][:])

            xr_i = projT(rwi, xi, 8, bxi, "xr_ia", eng="dve")

            # ia o-projection interleaved with ta AV stage: avstage PE items
            # fill the po copy-throughput gaps, and oproj_ta starts sooner.
            po_all_i = work.tile([128, 8 * N], CD, tag="po_all", bufs=2,
                                 name="po_all_i")
            oT_t = ps_oT.tile([128, N], f32, tag="oT", name="oT_t")
            po_dt = [0]

            def drain_po(k):
                while k > 0 and po_dt[0] < 8:
                    po_step("ia", po_all_i, onorm_i, xr_i, po_dt[0])
                    po_dt[0] += 1
                    k -= 1

            pend = []
            for it in range(NT):
                en, apad = ta_s1[it]
                eTp = attn_transp("ta", it, en, apad)
                pend.append((it,) + attn_ecopy("ta", it, eTp,
                                               apad is not None))
                drain_po(2)
                if len(pend) >= 2:
                    pit, peT, papr = pend.pop(0)
                    attn_av("ta", pit, peT, papr, v_t, oT_t)
            for pit, peT, papr in pend:
                attn_av("ta", pit, peT, papr, v_t, oT_t)
            drain_po(8)

            onorm_t = work.tile([128, N], CD, tag="onorm", bufs=2,
                                name="onorm_t")
            nc.vector.tensor_scalar_add(onorm_t[:], oT_t[:], sb["ta"]["bv"][:])
            xr_t = projT(rwt, xt, 6, bxt, "xr_ta", eng="dve")
            oproj("ta", onorm_t, xr_t)

    nc.compile()
    return nc


def _masks(cd):
    i = np.arange(N)
    length = np.minimum(N, i + 34) - np.maximum(0, i - 32)
    npad = np.maximum(0, WINDOW - length)
    mB = np.full((128, MW), -30000.0)
    for it in range(NT):
        gi = 128 * it + np.arange(128)[:, None]
        j = JT0[it] * 128 + np.arange(W_IT[it])[None, :]
        inband = (j >= gi - 32) & (j <= gi + 33)
        mB[:, MOFF[it]:MOFF[it] + W_IT[it]] = np.where(inband, 0.0, -30000.0)
    lnp = np.full((128, 2), -30000.0)
    for col, it in ((0, 0), (1, 3)):
        npv = npad[it * 128:(it + 1) * 128]
        lnp[:, col] = np.where(npv > 0, np.log(np.maximum(npv, 1)), -30000.0)
    return _host_cd(mB, cd), lnp.astype(np.float32)


def kernel(**inputs):
    from concourse.bass_utils import run_bass_kernel_spmd

    cd = COMPUTE_DTYPE
    if cd not in _prog_cache:
        _prog_cache[cd] = _build_program(cd)
    nc = _prog_cache[cd]

    f8 = lambda x: np.asarray(x, dtype=np.float64)
    images = f8(inputs["images"])[0]        # [N, 1024]
    caps = f8(inputs["capitions"])[0]       # [N, 768]
    ip_w, ip_b = f8(inputs["ip_w"]), f8(inputs["ip_b"])
    tp_w, tp_b = f8(inputs["tp_w"]), f8(inputs["tp_b"])

    sc = 1.0 / math.sqrt(DH)
    mB, lnp = _masks(cd)
    idn = np.eye(128)

    def pmaj(wT):
        """[C, X] -> partition-major [128, (C/128)*X]."""
        c, x = wT.shape
        return wT.reshape(c // 128, 128, x).transpose(1, 0, 2) \
                 .reshape(128, (c // 128) * x)

    xi_p = _host_cd(pmaj(images.T), cd)
    xt_p = _host_cd(pmaj(caps.T), cd)

    in_maps = []
    for h in range(NCORES):
        sl = slice(h * DH, (h + 1) * DH)
        ordh = [h] + [x for x in range(NCORES) if x != h]
        P = {}  # per-mod prepared pieces
        for m, pw, pb, cw, cb in (("ia", ip_w, ip_b, tp_w, tp_b),
                                  ("ta", tp_w, tp_b, ip_w, ip_b)):
            qw, qb = f8(inputs[f"{m}_qw"]), f8(inputs[f"{m}_qb"])
            kw, kb = f8(inputs[f"{m}_kw"]), f8(inputs[f"{m}_kb"])
            vw, vb = f8(inputs[f"{m}_vw"]), f8(inputs[f"{m}_vb"])
            ow = f8(inputs[f"{m}_ow"])
            cpr = vw[sl] @ cb
            woT = ow[:, sl].T
            P[m] = dict(
                wq=pmaj(((qw[sl] @ pw) * sc).T),
                bq=((qw[sl] @ pb + qb[sl]) * sc),
                wk=pmaj((kw[sl] @ cw).T),
                bk=(kw[sl] @ cb + kb[sl]),
                wv=pmaj((vw[sl] @ cw).T),
                bv=(cpr + vb[sl]),
                ngc=-cpr,
                kbc=kb[sl],
                wo=np.concatenate(
                    [woT[:, o * 128:(o + 1) * 128] for o in ordh], axis=1),
            )
        pf32 = np.stack([P["ia"]["bq"], P["ia"]["bk"], P["ia"]["bv"],
                         P["ta"]["bq"], P["ta"]["bk"], P["ta"]["bv"],
                         ip_b[sl], tp_b[sl]], axis=1)
        pf32 = np.concatenate([pf32, lnp.astype(np.float64)], axis=1)
        pk16 = np.zeros((128, MW + 128 + 2 + 256))
        pk16[:, 0:MW] = mB
        pk16[:, MW:MW + 128] = idn
        pk16[:, MW + 128] = P["ia"]["kbc"]
        pk16[:, MW + 129] = P["ta"]["kbc"]
        pk16[0, MW + 130:MW + 258] = P["ia"]["ngc"]
        pk16[0, MW + 258:MW + 386] = P["ta"]["ngc"]
        im = {
            "g_pf32": pf32.astype(np.float32),
            "g_wq_ia": _host_cd(P["ia"]["wq"], cd),
            "g_xi": xi_p,
            "g_kv_ta": _host_cd(
                np.concatenate([P["ta"]["wk"], P["ta"]["wv"]], axis=1), cd),
            "g_wk_ia": _host_cd(P["ia"]["wk"], cd),
            "g_xt": xt_p,
            "g_pk16": _host_cd(pk16, cd),
            "g_qv_ia": _host_cd(
                np.concatenate([P["ta"]["wq"], P["ia"]["wv"]], axis=1), cd),
            "g_rw_ia": _host_cd(
                np.concatenate([pmaj(ip_w[sl].T), P["ia"]["wo"]], axis=1), cd),
            "g_rw_ta": _host_cd(
                np.concatenate([pmaj(tp_w[sl].T), P["ta"]["wo"]], axis=1), cd),
        }
        in_maps.append(im)

    res = run_bass_kernel_spmd(nc, in_maps, list(range(NCORES)))
    LAST_RESULT["res"] = res

    outs = []
    for m in ("ia", "ta"):
        acc = np.zeros((DM, N), dtype=np.float64)
        for h in range(NCORES):
            r = res.results[h]
            po = r[f"po_{m}"].astype(np.float64)           # [128, 8*N]
            po = po.reshape(128, 8, N).transpose(1, 0, 2).reshape(DM, N)
            ordh = [h] + [x for x in range(NCORES) if x != h]
            for j, o in enumerate(ordh):
                acc[o * 128:(o + 1) * 128] += po[j * 128:(j + 1) * 128]
        acc += f8(inputs["ia_ob" if m == "ia" else "ta_ob"])[:, None]
        outs.append(np.ascontiguousarray(acc.T[None]).astype(np.float32))
    return outs[0], outs[1]
